# revision 54
# baseline (speedup 1.0000x reference)
"""Trainium2 Bass kernel for an attention block (B=8, T=2048, D=K=V=1024).

Reference math (per batch element, sharded one per NeuronCore):
    Q = x @ Wq.T + bq ; K = x @ Wk.T + bk ; V = x @ Wv.T + bv
    logits[t,s] = Q[t] . K[s],  masked -inf for s > t (strict upper tri)
    probs = softmax(logits, axis=t) / sqrt(1024)     # softmax over QUERY axis
    out = x + probs @ V

Implementation notes:
  - Transposed layout: QT/KT are [k, t] (k on partitions) so logitsT = [s, t]
    comes straight out of the PE; softmax reduction (over t) is a free-axis
    reduction via the activation accumulator.
  - QK path runs in bf16 (full PE rate, enables narrow diagonal tiles with no
    f32r small-free-dim penalty). V path runs in fp8e4m3 with the DoubleRow
    perf mode (2 contraction k-tiles per instruction, 2x rate); Wv is scaled
    by 16 before the fp8 cast to avoid subnormals, un-done in the r scaling.
    Q/K must NOT use fp8: sustained DoubleRow through phase 2 trips DVFS
    (~20% PE clock drop for the rest of the run) — the short V burst is fine.
  - P = exp(logits) is written directly to a triangular ragged SBUF buffer
    (row sv holds t-chunks tg=sv..15), so phase 4 reads it as matmul lhsT with
    no DRAM round trip; all xT blocks also stay resident (bf16 + fp8 copies).
  - Diagonal [128 s, 512 t] logits tiles are trimmed: the leading fully-masked
    128-col chunks are skipped; only the first remaining chunk needs the
    triangular -1e30 mask.
  - No max subtraction: exp saturates at fp32/bf16 max on HW; saturated
    columns give P/Z ~= 1 which is the correct dominant behavior.
  - Inputs are cast to bf16 before PE transposes (1.0 vs 1.5 cycles/row);
    Wq/Wk row-block transposes are interleaved with j=0's projection matmuls
    so the PE starts as soon as the first 512KB of weights lands.
  - DMA: ONE dma_start per 512KB tile — each issue costs ~630ns of engine
    time while its per-partition descriptors fan across all 16 HW engines;
    fine-grained splits are issue-rate-bound and stall the head.
  - Phase-4 xres/ost pools are reserved in the outer scope so their SBUF
    never overlaps phase-2/3 pools (otherwise the residual prefetch DMAs
    serialize behind the last V-proj reads).
"""

import os
import time

# Reset cores at runtime init: the PE clock can be left in a throttled
# power state by earlier workloads (observed ~20% slower matmuls); a core
# reset restores the full clock. Must be set before NRT initialization.
os.environ.setdefault("NEURON_RT_RESET_CORES", "1")

import numpy as np

import concourse.bass as bass
import concourse.bacc as bacc
import concourse.mybir as mybir
import concourse.tile as tile
from concourse.bass_utils import run_bass_kernel_spmd
from concourse.masks import make_identity

F32 = mybir.dt.float32
F32R = mybir.dt.float32r
BF16 = mybir.dt.bfloat16
FP8 = mybir.dt.float8e4
AF = mybir.ActivationFunctionType
DR = mybir.MatmulPerfMode.DoubleRow

P = 128          # partitions
T = 2048         # sequence length
D = 1024         # model dim
TB = 512         # t-block width
NTB = T // TB    # 4 t-blocks
DK = D // P      # 8 contraction subtiles
KO = D // P      # 8 k output tiles
SV = T // P      # 16 s tiles
NEG = -1.0e30
WV_SCALE = 16.0  # pre-scale on Wv before fp8 cast (avoids subnormals)


def _psb_off(sv):
    """Element offset of row sv in the ragged triangular P buffer.

    Row sv stores t-chunks tg = sv..15, each 128 wide."""
    return P * (16 * sv - (sv * (sv - 1)) // 2)


PSB_LEN = _psb_off(SV)  # 17408


def _transpose_weight_kt(nc, pools, w_ap, kt, dst, dst8=None, scale=1.0):
    """Transpose row-block kt of a [1024, 1024] DRAM weight into dst SBUF
    tile [128, 8, 1024] laid out as dst[d_inner, d_outer, k]. The f32 rows
    are cast to bf16 first so the PE transpose runs at 1.0 cycles/row
    (f32r transposes run at 1.5)."""
    wnat_pool, wcast_pool, psum_t, identity_bf = pools
    wnat = wnat_pool.tile([P, D], F32, name="wnat", tag="wnat")
    # ONE dma_start per row-block: a dma_start costs ~630ns of issue time on
    # the issuing engine, while its 128 per-partition descriptors fan out
    # across all 16 HW DMA engines — so big transfers, few issues.
    eng = nc.gpsimd if kt % 2 == 0 else nc.sync
    eng.dma_start(out=wnat, in_=w_ap[kt * P:(kt + 1) * P, :])
    wcast = wcast_pool.tile([P, D], BF16, name="wcast", tag="wcast")
    nc.vector.tensor_copy(out=wcast, in_=wnat)
    for dk in range(DK):
        pt = psum_t.tile([P, P], BF16, name="pt", tag="pt")
        nc.tensor.transpose(
            pt,
            wcast[:, dk * P:(dk + 1) * P],
            identity_bf,
        )
        if dst is not None:
            nc.vector.tensor_copy(
                out=dst[:, dk, kt * P:(kt + 1) * P], in_=pt)
        if dst8 is not None:
            # DVE (not Act): the PSUM slot must free fast; Act queues behind
            # bias/exp work and would hold the transpose pipeline hostage
            nc.vector.tensor_scalar_mul(
                dst8[:, dk, kt * P:(kt + 1) * P], pt, scale)


def _transpose_weight(nc, pools, w_ap, dst, dst8=None, scale=1.0):
    for kt in range(8):
        _transpose_weight_kt(nc, pools, w_ap, kt, dst, dst8, scale)


def _build_nc():
    nc = bacc.Bacc("TRN2", target_bir_lowering=False, debug=False, num_devices=8)

    x = nc.dram_tensor("x", [T, D], F32, kind="ExternalInput").ap()
    Wq = nc.dram_tensor("Wq", [D, D], F32, kind="ExternalInput").ap()
    bq = nc.dram_tensor("bq", [D], F32, kind="ExternalInput").ap()
    Wk = nc.dram_tensor("Wk", [D, D], F32, kind="ExternalInput").ap()
    bk = nc.dram_tensor("bk", [D], F32, kind="ExternalInput").ap()
    Wv = nc.dram_tensor("Wv", [D, D], F32, kind="ExternalInput").ap()
    bv = nc.dram_tensor("bv", [D], F32, kind="ExternalInput").ap()
    out = nc.dram_tensor("out", [T, D], F32, kind="ExternalOutput").ap()

    with tile.TileContext(nc) as tc:
        _kernel_body(nc, tc, x, Wq, bq, Wk, bk, Wv, bv, out)

    nc.compile()
    return nc


def _kernel_body(nc, tc, x, Wq, bq, Wk, bk, Wv, bv, out):
    from contextlib import ExitStack

    ctx = ExitStack()
    with ctx:
        consts = ctx.enter_context(tc.tile_pool(name="consts", bufs=1))
        wpool = ctx.enter_context(tc.tile_pool(name="wpool", bufs=2))
        w8pool = ctx.enter_context(tc.tile_pool(name="w8pool", bufs=1))
        ktpool = ctx.enter_context(tc.tile_pool(name="ktpool", bufs=1))
        psum_t = ctx.enter_context(tc.tile_pool(name="psum_t", bufs=2, space="PSUM"))
        psum_mm = ctx.enter_context(tc.tile_pool(name="psum_mm", bufs=6, space="PSUM"))
        # phase-4 pools live in the outer scope: if they shared SBUF with the
        # phase-2/3 pools, the xres prefetch DMAs would serialize behind the
        # last V-proj reads of that SBUF (observed as a 4us+ PE stall plus a
        # p-state ramp penalty at the phase-3/4 boundary)
        ost_pool = ctx.enter_context(tc.tile_pool(name="ost", bufs=2))
        xres_pool = ctx.enter_context(tc.tile_pool(name="xres", bufs=3))

        # ---- constants ----
        # identity first: it gates every PE transpose at kernel start
        id_f32 = consts.tile([P, P], F32, name="id_f32")
        make_identity(nc, id_f32)
        identity_bf = consts.tile([P, P], BF16, name="identity_bf")
        nc.vector.tensor_copy(out=identity_bf, in_=id_f32)

        # persistent KT [k_inner, k_outer, s] (bf16)
        KT = ktpool.tile([P, KO, T], BF16, name="KT", tag="big")

        # triangular ragged P buffer: row sv holds chunks tg=sv..15
        P_sb = consts.tile([P, PSB_LEN], BF16, name="P_sb")

        with (
            tc.tile_pool(name="wnat", bufs=2) as wnat_pool,
            tc.tile_pool(name="wcast", bufs=2) as wcast_pool,
            tc.tile_pool(name="xnat", bufs=2) as xnat_pool,
            tc.tile_pool(name="xcast", bufs=4) as xcast_pool,
            tc.tile_pool(name="xtp", bufs=3) as xtp,
            tc.tile_pool(name="x8p", bufs=4) as x8p,
            tc.tile_pool(name="qtp", bufs=1) as qtp,
        ):
            tpools = (wnat_pool, wcast_pool, psum_t, identity_bf)
            xT_blks = [None] * NTB
            xT8_blks = [None] * NTB

            def make_xT_blk(j):
                xT_blk = xtp.tile([P, DK, TB], BF16, name="xT_blk", tag="xT")
                xT8_blk = x8p.tile([P, DK, TB], FP8, name="xT8_blk", tag="xT8")
                xcasts = []
                for ts_ in range(TB // P):
                    t0 = j * TB + ts_ * P
                    xnat = xnat_pool.tile([P, D], F32, name="xnat", tag="xnat")
                    xcast = xcast_pool.tile([P, D], BF16, name="xcast",
                                            tag="xcast")
                    # one dma_start per tile (see _transpose_weight_kt); bf16
                    # cast before the transpose: bf16 transposes run 1.5x
                    # faster on the PE
                    nc.gpsimd.dma_start(out=xnat, in_=x[t0:t0 + P, :])
                    nc.vector.tensor_copy(out=xcast, in_=xnat)
                    xcasts.append(xcast)
                order = [(dk, ts_) for ts_ in range(TB // P)
                         for dk in range(DK)]
                for dk, ts_ in order:
                    pt = psum_t.tile([P, P], BF16, name="pt", tag="pt")
                    nc.tensor.transpose(
                        pt,
                        xcasts[ts_][:, dk * P:(dk + 1) * P],
                        identity_bf,
                    )
                    nc.vector.tensor_copy(
                        out=xT_blk[:, dk, ts_ * P:(ts_ + 1) * P], in_=pt
                    )
                    # fp8 cast reads the SBUF bf16 copy, not PSUM, on Act:
                    # gpsimd was tried and delays its own DMA issues; DVE
                    # would delay the transpose-copy chain
                    nc.scalar.activation(
                        xT8_blk[:, dk, ts_ * P:(ts_ + 1) * P],
                        xT_blk[:, dk, ts_ * P:(ts_ + 1) * P],
                        AF.Identity,
                    )
                xT_blks[j] = xT_blk
                xT8_blks[j] = xT8_blk

            # x transposes for blocks 0,1 run first: x tiles arrive long
            # before the full weight matrices, keeping the PE busy early.
            make_xT_blk(0)

            # triangular mask [128, 128]: valid (0.0) iff f >= p, else -1e30
            tri_mask = consts.tile([P, P], BF16, name="tri_mask")
            nc.gpsimd.memset(tri_mask, 0.0)
            nc.gpsimd.affine_select(
                out=tri_mask, in_=tri_mask,
                compare_op=mybir.AluOpType.is_ge,
                fill=NEG,
                base=0,
                pattern=[[1, P]],
                channel_multiplier=-1,
            )

            # biases: bq/bk striped [128, 8] (per-partition, k-major);
            # bv broadcast to all partitions [128, 1024], pre-scaled by 16
            # to match the fp8 Wv scaling (un-done via rtile /16).
            bq_sb = consts.tile([P, KO], F32, name="bq_sb")
            nc.sync.dma_start(out=bq_sb, in_=bq.rearrange("(o p) -> p o", p=P))
            bk_sb = consts.tile([P, KO], F32, name="bk_sb")
            nc.sync.dma_start(out=bk_sb, in_=bk.rearrange("(o p) -> p o", p=P))
            bv_sb = consts.tile([P, D], BF16, name="bv_sb")
            bv_bcast = bass.AP(tensor=bv.tensor, offset=bv.offset,
                               ap=[[0, P], [1, D]])
            nc.gpsimd.dma_start(out=bv_sb, in_=bv_bcast)
            nc.vector.tensor_scalar_mul(bv_sb, bv_sb, WV_SCALE)

            Zacc = consts.tile([P, SV, NTB], F32, name="Zacc")
            nc.vector.memset(Zacc, 0.0)
            rtile = consts.tile([P, SV], F32, name="rtile")

            # ---- phase 1+2 fused: weight transposes for Q, K are
            # interleaved with j=0's projection matmuls — matmul (ko, dk)
            # only needs weight row-block kt=ko, so the PE starts projecting
            # after the first 512KB of Wq arrives instead of all 4MB.
            # NOTE: Q/K projections must stay bf16 — fp8 DoubleRow through
            # phase 2 trips DVFS mid-run (~20% PE clock drop, re-confirmed
            # with per-run core resets); the short fp8 V-proj burst is fine.
            WqT = wpool.tile([P, DK, D], BF16, name="WqT", tag="W")
            WkT = wpool.tile([P, DK, D], BF16, name="WkT", tag="W")
            WvT8 = w8pool.tile([P, DK, D], FP8, name="WvT8", tag="W8")

            # ---- phase 2: fused x-transpose + QT/KT + logits + exp sweep ----
            for j in range(NTB):
                xT_blk = xT_blks[j]

                # QT block [k_inner, k_outer, t(512)]
                qt_blk = qtp.tile([P, KO, TB], BF16, name="qt_blk", tag="qt")
                for ko in range(KO):
                    if j == 0:
                        _transpose_weight_kt(nc, tpools, Wq, ko, WqT)
                    ps = psum_mm.tile([P, TB], F32, name="ps_q", tag="mm")
                    for dk in range(DK):
                        nc.tensor.matmul(
                            ps,
                            lhsT=WqT[:, dk, ko * P:(ko + 1) * P],
                            rhs=xT_blk[:, dk, :],
                            start=(dk == 0),
                            stop=(dk == DK - 1),
                        )
                    nc.scalar.activation(
                        qt_blk[:, ko, :], ps, AF.Identity,
                        bias=bq_sb[:, ko:ko + 1],
                    )

                # KT block
                for ko in range(KO):
                    if j == 0:
                        _transpose_weight_kt(nc, tpools, Wk, ko, WkT)
                    ps = psum_mm.tile([P, TB], F32, name="ps_k", tag="mm")
                    for dk in range(DK):
                        nc.tensor.matmul(
                            ps,
                            lhsT=WkT[:, dk, ko * P:(ko + 1) * P],
                            rhs=xT_blk[:, dk, :],
                            start=(dk == 0),
                            stop=(dk == DK - 1),
                        )
                    nc.scalar.activation(
                        KT[:, ko, j * TB:(j + 1) * TB], ps, AF.Identity,
                        bias=bk_sb[:, ko:ko + 1],
                    )

                if j == 0:
                    make_xT_blk(1)

                def logits_tile(sv):
                    oi = sv - 4 * j
                    w = TB if oi < 0 else (4 - oi) * P
                    toff = TB - w
                    ps = psum_mm.tile([P, TB], F32, name="ps_l", tag="mm")
                    for ko in range(KO):
                        nc.tensor.matmul(
                            ps[:, :w],
                            lhsT=KT[:, ko, sv * P:(sv + 1) * P],
                            rhs=qt_blk[:, ko, toff:],
                            start=(ko == 0),
                            stop=(ko == KO - 1),
                        )
                    if oi >= 0:
                        nc.vector.tensor_add(
                            out=ps[:, :P], in0=ps[:, :P], in1=tri_mask)
                    # P row sv, chunk position (t-chunk index - sv)
                    pos = _psb_off(sv) + (4 * j + (TB - w) // P - sv) * P
                    nc.scalar.activation(
                        P_sb[:, pos:pos + w], ps[:, :w], AF.Exp,
                        accum_out=Zacc[:, sv, j:j + 1],
                    )

                # next blocks' x transposes and the Wv transposes are emitted
                # mid-block so the PE reaches them long after their DMAs were
                # issued (no boundary stall), hidden between logits tiles
                logits_order = list(range(4 * (j + 1)))
                split = max(0, len(logits_order) - 4)
                for sv in logits_order[:split]:
                    logits_tile(sv)
                if j + 2 < NTB:
                    make_xT_blk(j + 2)
                if j == 1:
                    _transpose_weight(nc, tpools, Wv, None, dst8=WvT8,
                                      scale=WV_SCALE)
                for sv in logits_order[split:]:
                    logits_tile(sv)

            # ---- Z -> R = 1/(32 * 16 * Z) (the 16 un-does the Wv scale) ----
            zsum = consts.tile([P, SV], F32, name="zsum")
            nc.vector.reduce_sum(out=zsum, in_=Zacc, axis=mybir.AxisListType.X)
            nc.vector.reciprocal(rtile, zsum)
            nc.vector.tensor_scalar_mul(rtile, rtile, 1.0 / (32.0 * WV_SCALE))

            # ---- phases 3+4 interleaved ----
            # phase 3: V' = (x @ (16 Wv).T + 16 bv) / (512 Z), written into
            # Vp (reuses KT's SBUF slot, free after phase 2), fp8 DoubleRow.
            # phase 4: read = P^T . V', out = x + read. PV row-block i only
            # needs Vp[0..i], so it is emitted right after V-proj block
            # ceil((i+1)/4); this removes the phase boundary stall (and its
            # p-state ramp penalty) and warms the xres/ost pipeline early.
            Vp = ktpool.tile([P, SV, D], BF16, name="Vp", tag="big")

            def vproj_block(j):
                xT8_blk = xT8_blks[j]
                for si in range(TB // P):
                    sv = j * 4 + si
                    for h in range(D // TB):
                        ps = psum_mm.tile([P, TB], F32, name="ps_v", tag="mm")
                        for dp in range(DK // 2):
                            nc.tensor.matmul(
                                ps,
                                lhsT=xT8_blk[:, 2 * dp:2 * dp + 2,
                                             si * P:(si + 1) * P],
                                rhs=WvT8[:, 2 * dp:2 * dp + 2,
                                         h * TB:(h + 1) * TB],
                                start=(dp == 0),
                                stop=(dp == DK // 2 - 1),
                                perf_mode=DR,
                            )
                        # bias add on DVE (drains PSUM fast), r-scale on the
                        # scalar engine: with DVE alone, PSUM recycling (not
                        # the PE) paces this phase
                        nc.vector.tensor_add(
                            out=Vp[:, sv, h * TB:(h + 1) * TB],
                            in0=ps,
                            in1=bv_sb[:, h * TB:(h + 1) * TB],
                        )
                        nc.scalar.activation(
                            Vp[:, sv, h * TB:(h + 1) * TB],
                            Vp[:, sv, h * TB:(h + 1) * TB],
                            AF.Identity,
                            scale=rtile[:, sv:sv + 1],
                        )

            def pv_block(i):
                xres = xres_pool.tile([P, D], F32, name="xres", tag="xres")
                nc.gpsimd.dma_start(out=xres, in_=x[i * P:(i + 1) * P, :])
                ost = ost_pool.tile([P, D], F32, name="ost", tag="ost")
                for h in range(D // TB):
                    ps = psum_mm.tile([P, TB], F32, name="ps_o", tag="mm")
                    for svv in range(i + 1):
                        nc.tensor.matmul(
                            ps,
                            lhsT=P_sb[:, _psb_off(svv) + (i - svv) * P:
                                      _psb_off(svv) + (i - svv + 1) * P],
                            rhs=Vp[:, svv, h * TB:(h + 1) * TB],
                            start=(svv == 0),
                            stop=(svv == i),
                        )
                    nc.vector.tensor_add(
                        out=ost[:, h * TB:(h + 1) * TB],
                        in0=ps,
                        in1=xres[:, h * TB:(h + 1) * TB],
                    )
                    # store each half as soon as its residual add lands
                    nc.sync.dma_start(
                        out=out[i * P:(i + 1) * P, h * TB:(h + 1) * TB],
                        in_=ost[:, h * TB:(h + 1) * TB])

            # sequential: interleaving PV blocks between V-proj blocks was
            # tried and cost ~70us (Vp sub-tile write/read hazards serialize,
            # and spreading the fp8 bursts keeps the clock throttled longer);
            # descending PV order was also tried (shorter tail) but turns the
            # phase boundary into a full barrier — ascending wins.
            for j in range(NTB):
                vproj_block(j)
            for i in range(SV):
                pv_block(i)


_NC_CACHE = None


def _get_nc():
    global _NC_CACHE
    if _NC_CACHE is None:
        _NC_CACHE = _build_nc()
    return _NC_CACHE


def kernel(minibatch, Wq, bq, Wk, bk, Wv, bv):
    minibatch = np.asarray(minibatch, dtype=np.float32)
    Wq = np.asarray(Wq, dtype=np.float32)
    bq = np.asarray(bq, dtype=np.float32)
    Wk = np.asarray(Wk, dtype=np.float32)
    bk = np.asarray(bk, dtype=np.float32)
    Wv = np.asarray(Wv, dtype=np.float32)
    bv = np.asarray(bv, dtype=np.float32)

    nc = _get_nc()
    B = minibatch.shape[0]
    in_maps = [
        {
            "x": np.ascontiguousarray(minibatch[i]),
            "Wq": Wq, "bq": bq, "Wk": Wk, "bk": bk, "Wv": Wv, "bv": bv,
        }
        for i in range(B)
    ]
    last_err = None
    for _attempt in range(3):
        try:
            res = run_bass_kernel_spmd(nc, in_maps, core_ids=list(range(B)))
            break
        except Exception as e:  # transient device errors (e.g. NRT_EXEC_UNIT_UNRECOVERABLE)
            last_err = e
            time.sleep(2.0)
    else:
        raise last_err
    return np.stack([res.results[i]["out"] for i in range(B)], axis=0)


# revision 56
# speedup vs baseline: 1.0399x; 1.0399x over previous
"""Trainium2 Bass kernel for an attention block (B=8, T=2048, D=K=V=1024).

Reference math (per batch element, sharded one per NeuronCore):
    Q = x @ Wq.T + bq ; K = x @ Wk.T + bk ; V = x @ Wv.T + bv
    logits[t,s] = Q[t] . K[s],  masked -inf for s > t (strict upper tri)
    probs = softmax(logits, axis=t) / sqrt(1024)     # softmax over QUERY axis
    out = x + probs @ V

Implementation notes:
  - Transposed layout: QT/KT are [k, t] (k on partitions) so logitsT = [s, t]
    comes straight out of the PE; softmax reduction (over t) is a free-axis
    reduction via the activation accumulator.
  - QK path runs in bf16 (full PE rate, enables narrow diagonal tiles with no
    f32r small-free-dim penalty). V path runs in fp8e4m3 with the DoubleRow
    perf mode (2 contraction k-tiles per instruction, 2x rate); Wv is scaled
    by 16 before the fp8 cast to avoid subnormals, un-done in the r scaling.
    Q/K must NOT use fp8: sustained DoubleRow through phase 2 trips DVFS
    (~20% PE clock drop for the rest of the run) — the short V burst is fine.
  - P = exp(logits) is written directly to a triangular ragged SBUF buffer
    (row sv holds t-chunks tg=sv..15), so phase 4 reads it as matmul lhsT with
    no DRAM round trip; all xT blocks also stay resident (bf16 + fp8 copies).
  - Diagonal [128 s, 512 t] logits tiles are trimmed: the leading fully-masked
    128-col chunks are skipped; only the first remaining chunk needs the
    triangular -1e30 mask.
  - No max subtraction: exp saturates at fp32/bf16 max on HW; saturated
    columns give P/Z ~= 1 which is the correct dominant behavior.
  - Inputs are cast to bf16 before PE transposes (1.0 vs 1.5 cycles/row);
    Wq/Wk row-block transposes are interleaved with j=0's projection matmuls
    so the PE starts as soon as the first 512KB of weights lands.
  - DMA: ONE dma_start per 512KB tile — each issue costs ~630ns of engine
    time while its per-partition descriptors fan across all 16 HW engines;
    fine-grained splits are issue-rate-bound and stall the head.
  - Phase-4 xres/ost pools are reserved in the outer scope so their SBUF
    never overlaps phase-2/3 pools (otherwise the residual prefetch DMAs
    serialize behind the last V-proj reads).
"""

import os
import time

# Reset cores at runtime init: the PE clock can be left in a throttled
# power state by earlier workloads (observed ~20% slower matmuls); a core
# reset restores the full clock. Must be set before NRT initialization.
os.environ.setdefault("NEURON_RT_RESET_CORES", "1")

import numpy as np

import concourse.bass as bass
import concourse.bacc as bacc
import concourse.mybir as mybir
import concourse.tile as tile
from concourse.bass_utils import run_bass_kernel_spmd
from concourse.masks import make_identity

F32 = mybir.dt.float32
F32R = mybir.dt.float32r
BF16 = mybir.dt.bfloat16
FP8 = mybir.dt.float8e4
AF = mybir.ActivationFunctionType
DR = mybir.MatmulPerfMode.DoubleRow

P = 128          # partitions
T = 2048         # sequence length
D = 1024         # model dim
TB = 512         # t-block width
NTB = T // TB    # 4 t-blocks
DK = D // P      # 8 contraction subtiles
KO = D // P      # 8 k output tiles
SV = T // P      # 16 s tiles
NEG = -1.0e30
WV_SCALE = 16.0  # pre-scale on Wv before fp8 cast (avoids subnormals)


def _psb_off(sv):
    """Element offset of row sv in the ragged triangular P buffer.

    Row sv stores t-chunks tg = sv..15, each 128 wide."""
    return P * (16 * sv - (sv * (sv - 1)) // 2)


PSB_LEN = _psb_off(SV)  # 17408


def _transpose_weight_kt(nc, pools, w_ap, kt, dst, dst8=None, scale=1.0):
    """Transpose row-block kt of a [1024, 1024] DRAM weight into dst SBUF
    tile [128, 8, 1024] laid out as dst[d_inner, d_outer, k]. The f32 rows
    are cast to bf16 first so the PE transpose runs at 1.0 cycles/row
    (f32r transposes run at 1.5)."""
    wnat_pool, wcast_pool, psum_t, identity_bf = pools
    wnat = wnat_pool.tile([P, D], F32, name="wnat", tag="wnat")
    # ONE dma_start per row-block: a dma_start costs ~630ns of issue time on
    # the issuing engine, while its 128 per-partition descriptors fan out
    # across all 16 HW DMA engines — so big transfers, few issues.
    eng = nc.gpsimd if kt % 2 == 0 else nc.sync
    eng.dma_start(out=wnat, in_=w_ap[kt * P:(kt + 1) * P, :])
    wcast = wcast_pool.tile([P, D], BF16, name="wcast", tag="wcast")
    nc.vector.tensor_copy(out=wcast, in_=wnat)
    for dk in range(DK):
        pt = psum_t.tile([P, P], BF16, name="pt", tag="pt")
        nc.tensor.transpose(
            pt,
            wcast[:, dk * P:(dk + 1) * P],
            identity_bf,
        )
        if dst is not None:
            nc.vector.tensor_copy(
                out=dst[:, dk, kt * P:(kt + 1) * P], in_=pt)
        if dst8 is not None:
            # DVE (not Act): the PSUM slot must free fast; Act queues behind
            # bias/exp work and would hold the transpose pipeline hostage
            nc.vector.tensor_scalar_mul(
                dst8[:, dk, kt * P:(kt + 1) * P], pt, scale)


def _transpose_weight(nc, pools, w_ap, dst, dst8=None, scale=1.0):
    for kt in range(8):
        _transpose_weight_kt(nc, pools, w_ap, kt, dst, dst8, scale)


def _build_nc():
    nc = bacc.Bacc("TRN2", target_bir_lowering=False, debug=False, num_devices=8)

    x = nc.dram_tensor("x", [T, D], F32, kind="ExternalInput").ap()
    Wq = nc.dram_tensor("Wq", [D, D], F32, kind="ExternalInput").ap()
    bq = nc.dram_tensor("bq", [D], F32, kind="ExternalInput").ap()
    Wk = nc.dram_tensor("Wk", [D, D], F32, kind="ExternalInput").ap()
    bk = nc.dram_tensor("bk", [D], F32, kind="ExternalInput").ap()
    Wv = nc.dram_tensor("Wv", [D, D], F32, kind="ExternalInput").ap()
    bv = nc.dram_tensor("bv", [D], F32, kind="ExternalInput").ap()
    out = nc.dram_tensor("out", [T, D], F32, kind="ExternalOutput").ap()

    with tile.TileContext(nc) as tc:
        _kernel_body(nc, tc, x, Wq, bq, Wk, bk, Wv, bv, out)

    nc.compile()
    return nc


def _kernel_body(nc, tc, x, Wq, bq, Wk, bk, Wv, bv, out):
    from contextlib import ExitStack

    ctx = ExitStack()
    with ctx:
        consts = ctx.enter_context(tc.tile_pool(name="consts", bufs=1))
        wpool = ctx.enter_context(tc.tile_pool(name="wpool", bufs=2))
        w8pool = ctx.enter_context(tc.tile_pool(name="w8pool", bufs=1))
        ktpool = ctx.enter_context(tc.tile_pool(name="ktpool", bufs=1))
        psum_t = ctx.enter_context(tc.tile_pool(name="psum_t", bufs=2, space="PSUM"))
        psum_mm = ctx.enter_context(tc.tile_pool(name="psum_mm", bufs=6, space="PSUM"))
        # phase-4 pools live in the outer scope: if they shared SBUF with the
        # phase-2/3 pools, the xres prefetch DMAs would serialize behind the
        # last V-proj reads of that SBUF (observed as a 4us+ PE stall plus a
        # p-state ramp penalty at the phase-3/4 boundary)
        ost_pool = ctx.enter_context(tc.tile_pool(name="ost", bufs=2))
        xres_pool = ctx.enter_context(tc.tile_pool(name="xres", bufs=3))

        # ---- constants ----
        # identity first: it gates every PE transpose at kernel start
        id_f32 = consts.tile([P, P], F32, name="id_f32")
        make_identity(nc, id_f32)
        identity_bf = consts.tile([P, P], BF16, name="identity_bf")
        nc.vector.tensor_copy(out=identity_bf, in_=id_f32)

        # persistent KT [k_inner, k_outer, s] (bf16)
        KT = ktpool.tile([P, KO, T], BF16, name="KT", tag="big")

        # triangular ragged P buffer: row sv holds chunks tg=sv..15
        P_sb = consts.tile([P, PSB_LEN], BF16, name="P_sb")

        with (
            tc.tile_pool(name="wnat", bufs=1) as wnat_pool,
            tc.tile_pool(name="wcast", bufs=2) as wcast_pool,
            tc.tile_pool(name="xnat", bufs=2) as xnat_pool,
            tc.tile_pool(name="xcast", bufs=2) as xcast_pool,
            tc.tile_pool(name="xtp", bufs=3) as xtp,
            tc.tile_pool(name="x8p", bufs=4) as x8p,
            tc.tile_pool(name="qtp", bufs=2) as qtp,
        ):
            tpools = (wnat_pool, wcast_pool, psum_t, identity_bf)
            xT_blks = [None] * NTB
            xT8_blks = [None] * NTB

            def make_xT_blk(j):
                xT_blk = xtp.tile([P, DK, TB], BF16, name="xT_blk", tag="xT")
                xT8_blk = x8p.tile([P, DK, TB], FP8, name="xT8_blk", tag="xT8")
                xcasts = []
                for ts_ in range(TB // P):
                    t0 = j * TB + ts_ * P
                    xnat = xnat_pool.tile([P, D], F32, name="xnat", tag="xnat")
                    xcast = xcast_pool.tile([P, D], BF16, name="xcast",
                                            tag="xcast")
                    # one dma_start per tile (see _transpose_weight_kt); bf16
                    # cast before the transpose: bf16 transposes run 1.5x
                    # faster on the PE
                    if j == 0 and ts_ == 0:
                        # very first tile: split off the first 128-col chunk
                        # so the first PE transpose starts ~2us earlier
                        nc.gpsimd.dma_start(out=xnat[:, :P],
                                            in_=x[t0:t0 + P, :P])
                        nc.gpsimd.dma_start(out=xnat[:, P:],
                                            in_=x[t0:t0 + P, P:])
                        nc.vector.tensor_copy(out=xcast[:, :P],
                                              in_=xnat[:, :P])
                        nc.vector.tensor_copy(out=xcast[:, P:],
                                              in_=xnat[:, P:])
                    else:
                        nc.gpsimd.dma_start(out=xnat, in_=x[t0:t0 + P, :])
                        nc.vector.tensor_copy(out=xcast, in_=xnat)
                    xcasts.append(xcast)
                order = [(dk, ts_) for ts_ in range(TB // P)
                         for dk in range(DK)]
                for dk, ts_ in order:
                    pt = psum_t.tile([P, P], BF16, name="pt", tag="pt")
                    nc.tensor.transpose(
                        pt,
                        xcasts[ts_][:, dk * P:(dk + 1) * P],
                        identity_bf,
                    )
                    nc.vector.tensor_copy(
                        out=xT_blk[:, dk, ts_ * P:(ts_ + 1) * P], in_=pt
                    )
                    # fp8 cast reads the SBUF bf16 copy, not PSUM, on Act:
                    # gpsimd was tried and delays its own DMA issues; DVE
                    # would delay the transpose-copy chain
                    nc.scalar.activation(
                        xT8_blk[:, dk, ts_ * P:(ts_ + 1) * P],
                        xT_blk[:, dk, ts_ * P:(ts_ + 1) * P],
                        AF.Identity,
                    )
                xT_blks[j] = xT_blk
                xT8_blks[j] = xT8_blk

            # x transposes for blocks 0,1 run first: x tiles arrive long
            # before the full weight matrices, keeping the PE busy early.
            make_xT_blk(0)

            # triangular mask [128, 128]: valid (0.0) iff f >= p, else -1e30
            tri_mask = consts.tile([P, P], BF16, name="tri_mask")
            nc.gpsimd.memset(tri_mask, 0.0)
            nc.gpsimd.affine_select(
                out=tri_mask, in_=tri_mask,
                compare_op=mybir.AluOpType.is_ge,
                fill=NEG,
                base=0,
                pattern=[[1, P]],
                channel_multiplier=-1,
            )

            # biases: bq/bk striped [128, 8] (per-partition, k-major);
            # bv broadcast to all partitions [128, 1024], pre-scaled by 16
            # to match the fp8 Wv scaling (un-done via rtile /16).
            bq_sb = consts.tile([P, KO], F32, name="bq_sb")
            nc.sync.dma_start(out=bq_sb, in_=bq.rearrange("(o p) -> p o", p=P))
            bk_sb = consts.tile([P, KO], F32, name="bk_sb")
            nc.sync.dma_start(out=bk_sb, in_=bk.rearrange("(o p) -> p o", p=P))
            bv_sb = consts.tile([P, D], BF16, name="bv_sb")
            bv_bcast = bass.AP(tensor=bv.tensor, offset=bv.offset,
                               ap=[[0, P], [1, D]])
            nc.gpsimd.dma_start(out=bv_sb, in_=bv_bcast)
            nc.vector.tensor_scalar_mul(bv_sb, bv_sb, WV_SCALE)

            Zacc = consts.tile([P, SV, NTB], F32, name="Zacc")
            nc.vector.memset(Zacc, 0.0)
            rtile = consts.tile([P, SV], F32, name="rtile")

            # ---- phase 1+2 fused: weight transposes for Q, K are
            # interleaved with j=0's projection matmuls — matmul (ko, dk)
            # only needs weight row-block kt=ko, so the PE starts projecting
            # after the first 512KB of Wq arrives instead of all 4MB.
            # NOTE: Q/K projections must stay bf16 — fp8 DoubleRow through
            # phase 2 trips DVFS mid-run (~20% PE clock drop, re-confirmed
            # with per-run core resets); the short fp8 V-proj burst is fine.
            WqT = wpool.tile([P, DK, D], BF16, name="WqT", tag="W")
            WkT = wpool.tile([P, DK, D], BF16, name="WkT", tag="W")
            WvT8 = w8pool.tile([P, DK, D], FP8, name="WvT8", tag="W8")

            # ---- phase 2: fused x-transpose + QT/KT + logits + exp sweep ----
            for j in range(NTB):
                xT_blk = xT_blks[j]

                # QT block [k_inner, k_outer, t(512)]
                qt_blk = qtp.tile([P, KO, TB], BF16, name="qt_blk", tag="qt")
                for ko in range(KO):
                    if j == 0:
                        _transpose_weight_kt(nc, tpools, Wq, ko, WqT)
                    ps = psum_mm.tile([P, TB], F32, name="ps_q", tag="mm")
                    for dk in range(DK):
                        nc.tensor.matmul(
                            ps,
                            lhsT=WqT[:, dk, ko * P:(ko + 1) * P],
                            rhs=xT_blk[:, dk, :],
                            start=(dk == 0),
                            stop=(dk == DK - 1),
                        )
                    nc.scalar.activation(
                        qt_blk[:, ko, :], ps, AF.Identity,
                        bias=bq_sb[:, ko:ko + 1],
                    )

                # KT block
                for ko in range(KO):
                    if j == 0:
                        _transpose_weight_kt(nc, tpools, Wk, ko, WkT)
                    ps = psum_mm.tile([P, TB], F32, name="ps_k", tag="mm")
                    for dk in range(DK):
                        nc.tensor.matmul(
                            ps,
                            lhsT=WkT[:, dk, ko * P:(ko + 1) * P],
                            rhs=xT_blk[:, dk, :],
                            start=(dk == 0),
                            stop=(dk == DK - 1),
                        )
                    nc.scalar.activation(
                        KT[:, ko, j * TB:(j + 1) * TB], ps, AF.Identity,
                        bias=bk_sb[:, ko:ko + 1],
                    )

                if j == 0:
                    make_xT_blk(1)

                def logits_tile(sv):
                    oi = sv - 4 * j
                    w = TB if oi < 0 else (4 - oi) * P
                    toff = TB - w
                    ps = psum_mm.tile([P, TB], F32, name="ps_l", tag="mm")
                    for ko in range(KO):
                        nc.tensor.matmul(
                            ps[:, :w],
                            lhsT=KT[:, ko, sv * P:(sv + 1) * P],
                            rhs=qt_blk[:, ko, toff:],
                            start=(ko == 0),
                            stop=(ko == KO - 1),
                        )
                    if oi >= 0:
                        nc.vector.tensor_add(
                            out=ps[:, :P], in0=ps[:, :P], in1=tri_mask)
                    # P row sv, chunk position (t-chunk index - sv)
                    pos = _psb_off(sv) + (4 * j + (TB - w) // P - sv) * P
                    nc.scalar.activation(
                        P_sb[:, pos:pos + w], ps[:, :w], AF.Exp,
                        accum_out=Zacc[:, sv, j:j + 1],
                    )

                # next blocks' x transposes and the Wv transposes are emitted
                # mid-block so the PE reaches them long after their DMAs were
                # issued (no boundary stall), hidden between logits tiles
                logits_order = list(range(4 * (j + 1)))
                split = max(0, len(logits_order) - 4)
                for sv in logits_order[:split]:
                    logits_tile(sv)
                if j + 2 < NTB:
                    make_xT_blk(j + 2)
                if j == 1:
                    _transpose_weight(nc, tpools, Wv, None, dst8=WvT8,
                                      scale=WV_SCALE)
                for sv in logits_order[split:]:
                    logits_tile(sv)

            # ---- Z -> R = 1/(32 * 16 * Z) (the 16 un-does the Wv scale) ----
            zsum = consts.tile([P, SV], F32, name="zsum")
            nc.vector.reduce_sum(out=zsum, in_=Zacc, axis=mybir.AxisListType.X)
            nc.vector.reciprocal(rtile, zsum)
            nc.vector.tensor_scalar_mul(rtile, rtile, 1.0 / (32.0 * WV_SCALE))

            # ---- phases 3+4 interleaved ----
            # phase 3: V' = (x @ (16 Wv).T + 16 bv) / (512 Z), written into
            # Vp (reuses KT's SBUF slot, free after phase 2), fp8 DoubleRow.
            # phase 4: read = P^T . V', out = x + read. PV row-block i only
            # needs Vp[0..i], so it is emitted right after V-proj block
            # ceil((i+1)/4); this removes the phase boundary stall (and its
            # p-state ramp penalty) and warms the xres/ost pipeline early.
            Vp = ktpool.tile([P, SV, D], BF16, name="Vp", tag="big")

            def vproj_block(j):
                xT8_blk = xT8_blks[j]
                for si in range(TB // P):
                    sv = j * 4 + si
                    for h in range(D // TB):
                        ps = psum_mm.tile([P, TB], F32, name="ps_v", tag="mm")
                        for dp in range(DK // 2):
                            nc.tensor.matmul(
                                ps,
                                lhsT=xT8_blk[:, 2 * dp:2 * dp + 2,
                                             si * P:(si + 1) * P],
                                rhs=WvT8[:, 2 * dp:2 * dp + 2,
                                         h * TB:(h + 1) * TB],
                                start=(dp == 0),
                                stop=(dp == DK // 2 - 1),
                                perf_mode=DR,
                            )
                        # bias add on DVE (drains PSUM fast), r-scale on the
                        # scalar engine: with DVE alone, PSUM recycling (not
                        # the PE) paces this phase
                        nc.vector.tensor_add(
                            out=Vp[:, sv, h * TB:(h + 1) * TB],
                            in0=ps,
                            in1=bv_sb[:, h * TB:(h + 1) * TB],
                        )
                        nc.scalar.activation(
                            Vp[:, sv, h * TB:(h + 1) * TB],
                            Vp[:, sv, h * TB:(h + 1) * TB],
                            AF.Identity,
                            scale=rtile[:, sv:sv + 1],
                        )

            def pv_block(i):
                xres = xres_pool.tile([P, D], F32, name="xres", tag="xres")
                nc.gpsimd.dma_start(out=xres, in_=x[i * P:(i + 1) * P, :])
                ost = ost_pool.tile([P, D], F32, name="ost", tag="ost")
                for h in range(D // TB):
                    ps = psum_mm.tile([P, TB], F32, name="ps_o", tag="mm")
                    for svv in range(i + 1):
                        nc.tensor.matmul(
                            ps,
                            lhsT=P_sb[:, _psb_off(svv) + (i - svv) * P:
                                      _psb_off(svv) + (i - svv + 1) * P],
                            rhs=Vp[:, svv, h * TB:(h + 1) * TB],
                            start=(svv == 0),
                            stop=(svv == i),
                        )
                    nc.vector.tensor_add(
                        out=ost[:, h * TB:(h + 1) * TB],
                        in0=ps,
                        in1=xres[:, h * TB:(h + 1) * TB],
                    )
                    # store each half as soon as its residual add lands
                    nc.sync.dma_start(
                        out=out[i * P:(i + 1) * P, h * TB:(h + 1) * TB],
                        in_=ost[:, h * TB:(h + 1) * TB])

            # sequential: interleaving PV blocks between V-proj blocks was
            # tried and cost ~70us (Vp sub-tile write/read hazards serialize,
            # and spreading the fp8 bursts keeps the clock throttled longer);
            # descending PV order was also tried (shorter tail) but turns the
            # phase boundary into a full barrier — ascending wins.
            for j in range(NTB):
                vproj_block(j)
            for i in range(SV):
                pv_block(i)


_NC_CACHE = None


def _get_nc():
    global _NC_CACHE
    if _NC_CACHE is None:
        _NC_CACHE = _build_nc()
    return _NC_CACHE


def kernel(minibatch, Wq, bq, Wk, bk, Wv, bv):
    minibatch = np.asarray(minibatch, dtype=np.float32)
    Wq = np.asarray(Wq, dtype=np.float32)
    bq = np.asarray(bq, dtype=np.float32)
    Wk = np.asarray(Wk, dtype=np.float32)
    bk = np.asarray(bk, dtype=np.float32)
    Wv = np.asarray(Wv, dtype=np.float32)
    bv = np.asarray(bv, dtype=np.float32)

    nc = _get_nc()
    B = minibatch.shape[0]
    in_maps = [
        {
            "x": np.ascontiguousarray(minibatch[i]),
            "Wq": Wq, "bq": bq, "Wk": Wk, "bk": bk, "Wv": Wv, "bv": bv,
        }
        for i in range(B)
    ]
    last_err = None
    for _attempt in range(3):
        try:
            res = run_bass_kernel_spmd(nc, in_maps, core_ids=list(range(B)))
            break
        except Exception as e:  # transient device errors (e.g. NRT_EXEC_UNIT_UNRECOVERABLE)
            last_err = e
            time.sleep(2.0)
    else:
        raise last_err
    return np.stack([res.results[i]["out"] for i in range(B)], axis=0)


# revision 57
# speedup vs baseline: 1.1695x; 1.1247x over previous
"""Trainium2 Bass kernel for an attention block (B=8, T=2048, D=K=V=1024).

Reference math (per batch element, sharded one per NeuronCore):
    Q = x @ Wq.T + bq ; K = x @ Wk.T + bk ; V = x @ Wv.T + bv
    logits[t,s] = Q[t] . K[s],  masked -inf for s > t (strict upper tri)
    probs = softmax(logits, axis=t) / sqrt(1024)     # softmax over QUERY axis
    out = x + probs @ V

Implementation notes:
  - Transposed layout: QT/KT are [k, t] (k on partitions) so logitsT = [s, t]
    comes straight out of the PE; softmax reduction (over t) is a free-axis
    reduction via the activation accumulator.
  - QK path runs in bf16 (full PE rate, enables narrow diagonal tiles with no
    f32r small-free-dim penalty). V path runs in fp8e4m3 with the DoubleRow
    perf mode (2 contraction k-tiles per instruction, 2x rate); Wv is scaled
    by 16 before the fp8 cast to avoid subnormals, un-done in the r scaling.
    Q/K must NOT use fp8: sustained DoubleRow through phase 2 trips DVFS
    (~20% PE clock drop for the rest of the run) — the short V burst is fine.
  - P = exp(logits) is written directly to a triangular ragged SBUF buffer
    (row sv holds t-chunks tg=sv..15), so phase 4 reads it as matmul lhsT with
    no DRAM round trip; all xT blocks also stay resident (bf16 + fp8 copies).
  - Diagonal [128 s, 512 t] logits tiles are trimmed: the leading fully-masked
    128-col chunks are skipped; only the first remaining chunk needs the
    triangular -1e30 mask.
  - No max subtraction: exp saturates at fp32/bf16 max on HW; saturated
    columns give P/Z ~= 1 which is the correct dominant behavior.
  - Inputs are cast to bf16 before PE transposes (1.0 vs 1.5 cycles/row);
    Wq/Wk row-block transposes are interleaved with j=0's projection matmuls
    so the PE starts as soon as the first 512KB of weights lands.
  - DMA: ONE dma_start per 512KB tile — each issue costs ~630ns of engine
    time while its per-partition descriptors fan across all 16 HW engines;
    fine-grained splits are issue-rate-bound and stall the head.
  - Phase-4 xres/ost pools are reserved in the outer scope so their SBUF
    never overlaps phase-2/3 pools (otherwise the residual prefetch DMAs
    serialize behind the last V-proj reads).
"""

import os
import time

# Reset cores at runtime init: the PE clock can be left in a throttled
# power state by earlier workloads (observed ~20% slower matmuls); a core
# reset restores the full clock. Must be set before NRT initialization.
os.environ.setdefault("NEURON_RT_RESET_CORES", "1")

import numpy as np

import concourse.bass as bass
import concourse.bacc as bacc
import concourse.mybir as mybir
import concourse.tile as tile
from concourse.bass_utils import run_bass_kernel_spmd
from concourse.masks import make_identity

F32 = mybir.dt.float32
F32R = mybir.dt.float32r
BF16 = mybir.dt.bfloat16
FP8 = mybir.dt.float8e4
AF = mybir.ActivationFunctionType
DR = mybir.MatmulPerfMode.DoubleRow

P = 128          # partitions
T = 2048         # sequence length
D = 1024         # model dim
TB = 512         # t-block width
NTB = T // TB    # 4 t-blocks
DK = D // P      # 8 contraction subtiles
KO = D // P      # 8 k output tiles
SV = T // P      # 16 s tiles
NEG = -1.0e30
WV_SCALE = 16.0  # pre-scale on Wv before fp8 cast (avoids subnormals)


def _psb_off(sv):
    """Element offset of row sv in the ragged triangular P buffer.

    Row sv stores t-chunks tg = sv..15, each 128 wide."""
    return P * (16 * sv - (sv * (sv - 1)) // 2)


PSB_LEN = _psb_off(SV)  # 17408


def _transpose_weight_kt(nc, pools, w_ap, kt, dst, dst8=None, scale=1.0):
    """Transpose row-block kt of a [1024, 1024] DRAM weight into dst SBUF
    tile [128, 8, 1024] laid out as dst[d_inner, d_outer, k]. The f32 rows
    are cast to bf16 first so the PE transpose runs at 1.0 cycles/row
    (f32r transposes run at 1.5)."""
    wnat_pool, wcast_pool, psum_t, identity_bf = pools
    wnat = wnat_pool.tile([P, D], F32, name="wnat", tag="wnat")
    # ONE dma_start per row-block: a dma_start costs ~630ns of issue time on
    # the issuing engine, while its 128 per-partition descriptors fan out
    # across all 16 HW DMA engines — so big transfers, few issues.
    eng = nc.gpsimd if kt % 2 == 0 else nc.sync
    eng.dma_start(out=wnat, in_=w_ap[kt * P:(kt + 1) * P, :])
    wcast = wcast_pool.tile([P, D], BF16, name="wcast", tag="wcast")
    nc.vector.tensor_copy(out=wcast, in_=wnat)
    for dk in range(DK):
        pt = psum_t.tile([P, P], BF16, name="pt", tag="pt")
        nc.tensor.transpose(
            pt,
            wcast[:, dk * P:(dk + 1) * P],
            identity_bf,
        )
        if dst is not None:
            nc.vector.tensor_copy(
                out=dst[:, dk, kt * P:(kt + 1) * P], in_=pt)
        if dst8 is not None:
            # DVE (not Act): the PSUM slot must free fast; Act queues behind
            # bias/exp work and would hold the transpose pipeline hostage
            nc.vector.tensor_scalar_mul(
                dst8[:, dk, kt * P:(kt + 1) * P], pt, scale)


def _transpose_weight(nc, pools, w_ap, dst, dst8=None, scale=1.0):
    for kt in range(8):
        _transpose_weight_kt(nc, pools, w_ap, kt, dst, dst8, scale)


def _build_nc():
    nc = bacc.Bacc("TRN2", target_bir_lowering=False, debug=False, num_devices=8)

    x = nc.dram_tensor("x", [T, D], F32, kind="ExternalInput").ap()
    Wq = nc.dram_tensor("Wq", [D, D], F32, kind="ExternalInput").ap()
    bq = nc.dram_tensor("bq", [D], F32, kind="ExternalInput").ap()
    Wk = nc.dram_tensor("Wk", [D, D], F32, kind="ExternalInput").ap()
    bk = nc.dram_tensor("bk", [D], F32, kind="ExternalInput").ap()
    Wv = nc.dram_tensor("Wv", [D, D], F32, kind="ExternalInput").ap()
    bv = nc.dram_tensor("bv", [D], F32, kind="ExternalInput").ap()
    out = nc.dram_tensor("out", [T, D], F32, kind="ExternalOutput").ap()

    with tile.TileContext(nc) as tc:
        _kernel_body(nc, tc, x, Wq, bq, Wk, bk, Wv, bv, out)

    nc.compile()
    return nc


def _kernel_body(nc, tc, x, Wq, bq, Wk, bk, Wv, bv, out):
    from contextlib import ExitStack

    ctx = ExitStack()
    with ctx:
        consts = ctx.enter_context(tc.tile_pool(name="consts", bufs=1))
        wpool = ctx.enter_context(tc.tile_pool(name="wpool", bufs=2))
        w8pool = ctx.enter_context(tc.tile_pool(name="w8pool", bufs=1))
        ktpool = ctx.enter_context(tc.tile_pool(name="ktpool", bufs=1))
        psum_t = ctx.enter_context(tc.tile_pool(name="psum_t", bufs=2, space="PSUM"))
        psum_mm = ctx.enter_context(tc.tile_pool(name="psum_mm", bufs=6, space="PSUM"))
        # phase-4 pools live in the outer scope: if they shared SBUF with the
        # phase-2/3 pools, the xres prefetch DMAs would serialize behind the
        # last V-proj reads of that SBUF (observed as a 4us+ PE stall plus a
        # p-state ramp penalty at the phase-3/4 boundary)
        ost_pool = ctx.enter_context(tc.tile_pool(name="ost", bufs=2))
        xres_pool = ctx.enter_context(tc.tile_pool(name="xres", bufs=3))

        # ---- constants ----
        # identity first: it gates every PE transpose at kernel start
        id_f32 = consts.tile([P, P], F32, name="id_f32")
        make_identity(nc, id_f32)
        identity_bf = consts.tile([P, P], BF16, name="identity_bf")
        nc.vector.tensor_copy(out=identity_bf, in_=id_f32)

        # persistent KT [k_inner, k_outer, s] (bf16)
        KT = ktpool.tile([P, KO, T], BF16, name="KT", tag="big")

        # triangular ragged P buffer: row sv holds chunks tg=sv..15
        P_sb = consts.tile([P, PSB_LEN], BF16, name="P_sb")

        with (
            tc.tile_pool(name="wnat", bufs=2) as wnat_pool,
            tc.tile_pool(name="wcast", bufs=2) as wcast_pool,
            tc.tile_pool(name="xnat", bufs=2) as xnat_pool,
            tc.tile_pool(name="xcast", bufs=4) as xcast_pool,
            tc.tile_pool(name="xtp", bufs=3) as xtp,
            tc.tile_pool(name="x8p", bufs=4) as x8p,
            tc.tile_pool(name="qtp", bufs=1) as qtp,
        ):
            tpools = (wnat_pool, wcast_pool, psum_t, identity_bf)
            xT_blks = [None] * NTB
            xT8_blks = [None] * NTB

            def make_xT_blk(j):
                xT_blk = xtp.tile([P, DK, TB], BF16, name="xT_blk", tag="xT")
                xT8_blk = x8p.tile([P, DK, TB], FP8, name="xT8_blk", tag="xT8")
                xcasts = []
                for ts_ in range(TB // P):
                    t0 = j * TB + ts_ * P
                    xnat = xnat_pool.tile([P, D], F32, name="xnat", tag="xnat")
                    xcast = xcast_pool.tile([P, D], BF16, name="xcast",
                                            tag="xcast")
                    # one dma_start per tile (see _transpose_weight_kt); bf16
                    # cast before the transpose: bf16 transposes run 1.5x
                    # faster on the PE
                    nc.gpsimd.dma_start(out=xnat, in_=x[t0:t0 + P, :])
                    nc.vector.tensor_copy(out=xcast, in_=xnat)
                    xcasts.append(xcast)
                order = [(dk, ts_) for ts_ in range(TB // P)
                         for dk in range(DK)]
                for dk, ts_ in order:
                    pt = psum_t.tile([P, P], BF16, name="pt", tag="pt")
                    nc.tensor.transpose(
                        pt,
                        xcasts[ts_][:, dk * P:(dk + 1) * P],
                        identity_bf,
                    )
                    nc.vector.tensor_copy(
                        out=xT_blk[:, dk, ts_ * P:(ts_ + 1) * P], in_=pt
                    )
                    # fp8 cast reads the SBUF bf16 copy, not PSUM, on Act:
                    # gpsimd was tried and delays its own DMA issues; DVE
                    # would delay the transpose-copy chain
                    nc.scalar.activation(
                        xT8_blk[:, dk, ts_ * P:(ts_ + 1) * P],
                        xT_blk[:, dk, ts_ * P:(ts_ + 1) * P],
                        AF.Identity,
                    )
                xT_blks[j] = xT_blk
                xT8_blks[j] = xT8_blk

            # x transposes for blocks 0,1 run first: x tiles arrive long
            # before the full weight matrices, keeping the PE busy early.
            make_xT_blk(0)

            # triangular mask [128, 128]: valid (0.0) iff f >= p, else -1e30
            tri_mask = consts.tile([P, P], BF16, name="tri_mask")
            nc.gpsimd.memset(tri_mask, 0.0)
            nc.gpsimd.affine_select(
                out=tri_mask, in_=tri_mask,
                compare_op=mybir.AluOpType.is_ge,
                fill=NEG,
                base=0,
                pattern=[[1, P]],
                channel_multiplier=-1,
            )

            # biases: bq/bk striped [128, 8] (per-partition, k-major);
            # bv broadcast to all partitions [128, 1024], pre-scaled by 16
            # to match the fp8 Wv scaling (un-done via rtile /16).
            bq_sb = consts.tile([P, KO], F32, name="bq_sb")
            nc.sync.dma_start(out=bq_sb, in_=bq.rearrange("(o p) -> p o", p=P))
            bk_sb = consts.tile([P, KO], F32, name="bk_sb")
            nc.sync.dma_start(out=bk_sb, in_=bk.rearrange("(o p) -> p o", p=P))
            bv_sb = consts.tile([P, D], BF16, name="bv_sb")
            bv_bcast = bass.AP(tensor=bv.tensor, offset=bv.offset,
                               ap=[[0, P], [1, D]])
            nc.gpsimd.dma_start(out=bv_sb, in_=bv_bcast)
            nc.vector.tensor_scalar_mul(bv_sb, bv_sb, WV_SCALE)

            Zacc = consts.tile([P, SV, NTB], F32, name="Zacc")
            nc.vector.memset(Zacc, 0.0)
            rtile = consts.tile([P, SV], F32, name="rtile")

            # ---- phase 1+2 fused: weight transposes for Q, K are
            # interleaved with j=0's projection matmuls — matmul (ko, dk)
            # only needs weight row-block kt=ko, so the PE starts projecting
            # after the first 512KB of Wq arrives instead of all 4MB.
            # NOTE: Q/K projections must stay bf16 — fp8 DoubleRow through
            # phase 2 trips DVFS mid-run (~20% PE clock drop, re-confirmed
            # with per-run core resets); the short fp8 V-proj burst is fine.
            WqT = wpool.tile([P, DK, D], BF16, name="WqT", tag="W")
            WkT = wpool.tile([P, DK, D], BF16, name="WkT", tag="W")
            WvT8 = w8pool.tile([P, DK, D], FP8, name="WvT8", tag="W8")

            # ---- phase 2: fused x-transpose + QT/KT + logits + exp sweep ----
            for j in range(NTB):
                xT_blk = xT_blks[j]

                # QT block [k_inner, k_outer, t(512)]
                qt_blk = qtp.tile([P, KO, TB], BF16, name="qt_blk", tag="qt")
                for ko in range(KO):
                    if j == 0:
                        _transpose_weight_kt(nc, tpools, Wq, ko, WqT)
                    ps = psum_mm.tile([P, TB], F32, name="ps_q", tag="mm")
                    for dk in range(DK):
                        nc.tensor.matmul(
                            ps,
                            lhsT=WqT[:, dk, ko * P:(ko + 1) * P],
                            rhs=xT_blk[:, dk, :],
                            start=(dk == 0),
                            stop=(dk == DK - 1),
                        )
                    nc.scalar.activation(
                        qt_blk[:, ko, :], ps, AF.Identity,
                        bias=bq_sb[:, ko:ko + 1],
                    )

                # KT block
                for ko in range(KO):
                    if j == 0:
                        _transpose_weight_kt(nc, tpools, Wk, ko, WkT)
                    ps = psum_mm.tile([P, TB], F32, name="ps_k", tag="mm")
                    for dk in range(DK):
                        nc.tensor.matmul(
                            ps,
                            lhsT=WkT[:, dk, ko * P:(ko + 1) * P],
                            rhs=xT_blk[:, dk, :],
                            start=(dk == 0),
                            stop=(dk == DK - 1),
                        )
                    nc.scalar.activation(
                        KT[:, ko, j * TB:(j + 1) * TB], ps, AF.Identity,
                        bias=bk_sb[:, ko:ko + 1],
                    )

                if j == 0:
                    make_xT_blk(1)

                def logits_tile(sv):
                    oi = sv - 4 * j
                    w = TB if oi < 0 else (4 - oi) * P
                    toff = TB - w
                    ps = psum_mm.tile([P, TB], F32, name="ps_l", tag="mm")
                    for ko in range(KO):
                        nc.tensor.matmul(
                            ps[:, :w],
                            lhsT=KT[:, ko, sv * P:(sv + 1) * P],
                            rhs=qt_blk[:, ko, toff:],
                            start=(ko == 0),
                            stop=(ko == KO - 1),
                        )
                    if oi >= 0:
                        nc.vector.tensor_add(
                            out=ps[:, :P], in0=ps[:, :P], in1=tri_mask)
                    # P row sv, chunk position (t-chunk index - sv)
                    pos = _psb_off(sv) + (4 * j + (TB - w) // P - sv) * P
                    nc.scalar.activation(
                        P_sb[:, pos:pos + w], ps[:, :w], AF.Exp,
                        accum_out=Zacc[:, sv, j:j + 1],
                    )

                # next blocks' x transposes and the Wv transposes are emitted
                # mid-block so the PE reaches them long after their DMAs were
                # issued (no boundary stall), hidden between logits tiles
                logits_order = list(range(4 * (j + 1)))
                split = max(0, len(logits_order) - 4)
                for sv in logits_order[:split]:
                    logits_tile(sv)
                if j + 2 < NTB:
                    make_xT_blk(j + 2)
                if j == 1:
                    _transpose_weight(nc, tpools, Wv, None, dst8=WvT8,
                                      scale=WV_SCALE)
                for sv in logits_order[split:]:
                    logits_tile(sv)

            # ---- Z -> R = 1/(32 * 16 * Z) (the 16 un-does the Wv scale) ----
            zsum = consts.tile([P, SV], F32, name="zsum")
            nc.vector.reduce_sum(out=zsum, in_=Zacc, axis=mybir.AxisListType.X)
            nc.vector.reciprocal(rtile, zsum)
            nc.vector.tensor_scalar_mul(rtile, rtile, 1.0 / (32.0 * WV_SCALE))

            # ---- phases 3+4 interleaved ----
            # phase 3: V' = (x @ (16 Wv).T + 16 bv) / (512 Z), written into
            # Vp (reuses KT's SBUF slot, free after phase 2), fp8 DoubleRow.
            # phase 4: read = P^T . V', out = x + read. PV row-block i only
            # needs Vp[0..i], so it is emitted right after V-proj block
            # ceil((i+1)/4); this removes the phase boundary stall (and its
            # p-state ramp penalty) and warms the xres/ost pipeline early.
            Vp = ktpool.tile([P, SV, D], BF16, name="Vp", tag="big")

            def vproj_block(j):
                xT8_blk = xT8_blks[j]
                for si in range(TB // P):
                    sv = j * 4 + si
                    for h in range(D // TB):
                        ps = psum_mm.tile([P, TB], F32, name="ps_v", tag="mm")
                        for dp in range(DK // 2):
                            nc.tensor.matmul(
                                ps,
                                lhsT=xT8_blk[:, 2 * dp:2 * dp + 2,
                                             si * P:(si + 1) * P],
                                rhs=WvT8[:, 2 * dp:2 * dp + 2,
                                         h * TB:(h + 1) * TB],
                                start=(dp == 0),
                                stop=(dp == DK // 2 - 1),
                                perf_mode=DR,
                            )
                        # bias add on DVE (drains PSUM fast), r-scale on the
                        # scalar engine: with DVE alone, PSUM recycling (not
                        # the PE) paces this phase
                        nc.vector.tensor_add(
                            out=Vp[:, sv, h * TB:(h + 1) * TB],
                            in0=ps,
                            in1=bv_sb[:, h * TB:(h + 1) * TB],
                        )
                        nc.scalar.activation(
                            Vp[:, sv, h * TB:(h + 1) * TB],
                            Vp[:, sv, h * TB:(h + 1) * TB],
                            AF.Identity,
                            scale=rtile[:, sv:sv + 1],
                        )

            def pv_block(i):
                xres = xres_pool.tile([P, D], F32, name="xres", tag="xres")
                nc.gpsimd.dma_start(out=xres, in_=x[i * P:(i + 1) * P, :])
                ost = ost_pool.tile([P, D], F32, name="ost", tag="ost")
                for h in range(D // TB):
                    ps = psum_mm.tile([P, TB], F32, name="ps_o", tag="mm")
                    for svv in range(i + 1):
                        nc.tensor.matmul(
                            ps,
                            lhsT=P_sb[:, _psb_off(svv) + (i - svv) * P:
                                      _psb_off(svv) + (i - svv + 1) * P],
                            rhs=Vp[:, svv, h * TB:(h + 1) * TB],
                            start=(svv == 0),
                            stop=(svv == i),
                        )
                    nc.vector.tensor_add(
                        out=ost[:, h * TB:(h + 1) * TB],
                        in0=ps,
                        in1=xres[:, h * TB:(h + 1) * TB],
                    )
                    # store each half as soon as its residual add lands
                    nc.sync.dma_start(
                        out=out[i * P:(i + 1) * P, h * TB:(h + 1) * TB],
                        in_=ost[:, h * TB:(h + 1) * TB])

            # sequential: interleaving PV blocks between V-proj blocks was
            # tried and cost ~70us (Vp sub-tile write/read hazards serialize,
            # and spreading the fp8 bursts keeps the clock throttled longer);
            # descending PV order was also tried (shorter tail) but turns the
            # phase boundary into a full barrier — ascending wins.
            for j in range(NTB):
                vproj_block(j)
            for i in range(SV):
                pv_block(i)


_NC_CACHE = None


def _get_nc():
    global _NC_CACHE
    if _NC_CACHE is None:
        _NC_CACHE = _build_nc()
    return _NC_CACHE


def kernel(minibatch, Wq, bq, Wk, bk, Wv, bv):
    minibatch = np.asarray(minibatch, dtype=np.float32)
    Wq = np.asarray(Wq, dtype=np.float32)
    bq = np.asarray(bq, dtype=np.float32)
    Wk = np.asarray(Wk, dtype=np.float32)
    bk = np.asarray(bk, dtype=np.float32)
    Wv = np.asarray(Wv, dtype=np.float32)
    bv = np.asarray(bv, dtype=np.float32)

    nc = _get_nc()
    B = minibatch.shape[0]
    in_maps = [
        {
            "x": np.ascontiguousarray(minibatch[i]),
            "Wq": Wq, "bq": bq, "Wk": Wk, "bk": bk, "Wv": Wv, "bv": bv,
        }
        for i in range(B)
    ]
    last_err = None
    for _attempt in range(3):
        try:
            res = run_bass_kernel_spmd(nc, in_maps, core_ids=list(range(B)))
            break
        except Exception as e:  # transient device errors (e.g. NRT_EXEC_UNIT_UNRECOVERABLE)
            last_err = e
            time.sleep(2.0)
    else:
        raise last_err
    return np.stack([res.results[i]["out"] for i in range(B)], axis=0)
